# revision 3
# baseline (speedup 1.0000x reference)
"""MoE layer kernel for 8 Trainium2 NeuronCores.

Strategy (expert parallelism, per sharding hint):
  - Router (logits/softmax/top-k), token dispatch and weighted combine run on
    host in fp64/fp32 (0.05% of total FLOPs).
  - Each of the 8 cores owns one expert: it receives that expert's routed
    tokens (transposed, capacity-padded) plus its w1/b1/w2/b2 and computes
    gelu(x @ w1 + b1) @ w2 + b2 for its tokens.
  - Device matmuls run in fp32r (full PE rate, ~1.5e-4 rel err); fp32
    accumulation in PSUM; bias+gelu fused on the scalar engine.
  - Host scatters the weighted per-expert outputs back (each token has
    exactly K=2 contributions).
"""

import sys

if "/opt/trn_rl_repo" not in sys.path:
    sys.path.insert(0, "/opt/trn_rl_repo")

import numpy as np

B, S, H = 2, 2048, 1024
E, KTOP, F = 8, 2, 4096
T = B * S
P = 128
ROUTER_AUX_COEF = 0.001
ROUTER_Z_COEF = 0.001

_PROGRAM_CACHE = {}


def _decompose(C):
    """Split C (multiple of 128) into pieces <=512, each a multiple of 128,
    as equal as possible (so pieces are >=256 whenever C allows)."""
    k = C // P
    n = (k + 3) // 4
    base, rem = divmod(k, n)
    return [(base + 1) * P] * rem + [base * P] * (n - rem)


def _build_program(C, subs, repeat=1):
    import concourse.tile as tile
    from concourse import bacc, mybir

    f32 = mybir.dt.float32
    f32r = mybir.dt.float32r
    GELU = mybir.ActivationFunctionType.Gelu
    IDENT = mybir.ActivationFunctionType.Identity

    nsub = len(subs)
    offs = np.concatenate([[0], np.cumsum(subs)]).astype(int)
    NH = H // P            # 8 h-tiles
    NF = F // P            # 32 f-tiles
    NFH = NF // 2          # 16 f-tiles per half

    nc = bacc.Bacc("TRN2", target_bir_lowering=False, debug=False, num_devices=8)

    XT_d = nc.dram_tensor("xt", [H, C], f32r, kind="ExternalInput").ap()
    W1_d = nc.dram_tensor("w1", [H, F], f32r, kind="ExternalInput").ap()
    W2_d = nc.dram_tensor("w2", [F, H], f32r, kind="ExternalInput").ap()
    B1_d = nc.dram_tensor("b1", [P, NF], f32, kind="ExternalInput").ap()
    B2_d = nc.dram_tensor("b2", [P, NH], f32, kind="ExternalInput").ap()
    YT_d = nc.dram_tensor("yt", [H, C], f32, kind="ExternalOutput").ap()

    xt_t = XT_d.rearrange("(n p) c -> n p c", p=P)   # [8, 128, C]
    w1_t = W1_d.rearrange("(n p) f -> n p f", p=P)   # [8, 128, 4096]
    w2_t = W2_d.rearrange("(n p) h -> n p h", p=P)   # [32, 128, 1024]
    yt_t = YT_d.rearrange("(n p) c -> n p c", p=P)   # [8, 128, C]

    with tile.TileContext(nc) as tc:
        with (
            tc.tile_pool(name="const", bufs=1) as const_pool,
            tc.tile_pool(name="xt", bufs=NH) as xt_pool,
            tc.tile_pool(name="y", bufs=NH) as y_pool,
            tc.tile_pool(name="mid", bufs=NFH) as mid_pool,
            tc.tile_pool(name="w1s", bufs=3) as w1_pool,
            tc.tile_pool(name="w2s", bufs=2) as w2_pool,
            tc.tile_pool(name="ps", bufs=min(2 * nsub, 8), space="PSUM") as ps_pool,
        ):
            for _rep in range(repeat):
                b1_sb = const_pool.tile([P, NF], f32, tag="b1")
                nc.sync.dma_start(b1_sb[:], B1_d[:, :])
                b2_sb = const_pool.tile([P, NH], f32, tag="b2")
                nc.sync.dma_start(b2_sb[:], B2_d[:, :])

                xt_sb = []
                for h0 in range(NH):
                    t = xt_pool.tile([P, C], f32r, tag="xt", name="xt_sb")
                    nc.sync.dma_start(t[:], xt_t[h0])
                    xt_sb.append(t)

                y_sb = [y_pool.tile([P, C], f32, tag="y", name="y_sb") for _ in range(NH)]

                for half in range(2):
                    # ---- phase A: midT[f0] = gelu(w1[:,f0].T @ X^T + b1) ----
                    mids = []
                    for fi in range(NFH):
                        f0 = half * NFH + fi
                        w1s = w1_pool.tile([P, NH * P], f32r, tag="w1s")
                        for h0 in range(NH):
                            nc.sync.dma_start(
                                w1s[:, h0 * P:(h0 + 1) * P],
                                w1_t[h0][:, f0 * P:(f0 + 1) * P],
                            )
                        pss = [ps_pool.tile([P, s], f32, tag="ps", name="ps") for s in subs]
                        for h0 in range(NH):
                            lhsT = w1s[:, h0 * P:(h0 + 1) * P]
                            for si in range(nsub):
                                nc.tensor.matmul(
                                    pss[si][:],
                                    lhsT,
                                    xt_sb[h0][:, offs[si]:offs[si + 1]],
                                    start=(h0 == 0),
                                    stop=(h0 == NH - 1),
                                )
                        mid = mid_pool.tile([P, C], f32r, tag="mid")
                        for si in range(nsub):
                            nc.scalar.activation(
                                mid[:, offs[si]:offs[si + 1]],
                                pss[si][:],
                                GELU,
                                bias=b1_sb[:, f0:f0 + 1],
                            )
                        mids.append(mid)

                    # ---- phase B: y[h0] (+)= w2[f-half, h0].T @ midT ----
                    for h0 in range(NH):
                        w2s = w2_pool.tile([P, NFH * P], f32r, tag="w2s")
                        for j in range(NFH):
                            f0 = half * NFH + j
                            nc.sync.dma_start(
                                w2s[:, j * P:(j + 1) * P],
                                w2_t[f0][:, h0 * P:(h0 + 1) * P],
                            )
                        pss = [ps_pool.tile([P, s], f32, tag="ps", name="ps") for s in subs]
                        for j in range(NFH):
                            lhsT = w2s[:, j * P:(j + 1) * P]
                            for si in range(nsub):
                                nc.tensor.matmul(
                                    pss[si][:],
                                    lhsT,
                                    mids[j][:, offs[si]:offs[si + 1]],
                                    start=(j == 0),
                                    stop=(j == NFH - 1),
                                )
                        for si in range(nsub):
                            ysl = y_sb[h0][:, offs[si]:offs[si + 1]]
                            if half == 0:
                                nc.scalar.activation(
                                    ysl, pss[si][:], IDENT, bias=b2_sb[:, h0:h0 + 1]
                                )
                            else:
                                nc.vector.tensor_add(ysl, ysl, pss[si][:])
                        if half == 1:
                            nc.sync.dma_start(yt_t[h0], y_sb[h0][:])

    nc.compile()
    return nc


def _get_program(C, subs, repeat=1):
    key = (C, tuple(subs), repeat)
    if key not in _PROGRAM_CACHE:
        _PROGRAM_CACHE[key] = _build_program(C, subs, repeat)
    return _PROGRAM_CACHE[key]


def _route(x, w_router):
    """Host router in fp64; returns fp32 probs/indices matching jax fp32
    top_k semantics (descending, ties -> lower index)."""
    logits = x.astype(np.float64) @ w_router.astype(np.float64)
    logits -= logits.max(axis=-1, keepdims=True)
    p = np.exp(logits)
    p /= p.sum(axis=-1, keepdims=True)
    p32 = p.astype(np.float32)
    idx = np.argsort(-p32, axis=-1, kind="stable")[:, :KTOP].astype(np.int32)
    route_probs = np.take_along_axis(p32, idx, axis=-1)
    return p, p32, idx, route_probs


def _aux_loss(p64, idx, route_probs):
    counts = np.bincount(idx.ravel(), minlength=E)
    f = counts.astype(np.float64) / T
    Pm = p64.mean(axis=0)
    lb = E * np.sum(f * Pm)
    rp = route_probs.astype(np.float64)
    m = rp.max(axis=-1)
    z = np.log(np.exp(rp - m[:, None]).sum(axis=-1)) + m
    zl = np.mean(z * z)
    return np.float32(lb * ROUTER_AUX_COEF + zl * ROUTER_Z_COEF)


def _dispatch(x, idx, route_probs):
    """Group token ids and weights by expert."""
    flat_e = idx.ravel()
    flat_tok = np.repeat(np.arange(T, dtype=np.int64), KTOP)
    flat_p = route_probs.ravel()
    order = np.argsort(flat_e, kind="stable")
    e_sorted = flat_e[order]
    tok_sorted = flat_tok[order]
    p_sorted = flat_p[order]
    counts = np.bincount(flat_e, minlength=E)
    starts = np.concatenate([[0], np.cumsum(counts)])
    toks = [tok_sorted[starts[e]:starts[e + 1]] for e in range(E)]
    ps = [p_sorted[starts[e]:starts[e + 1]] for e in range(E)]
    return toks, ps, counts


def _run_device(nc, in_maps):
    from concourse.bass_utils import run_bass_kernel_spmd

    return run_bass_kernel_spmd(nc, in_maps, list(range(E)))


def _make_in_maps(x, toks, w1, b1, w2, b2, C):
    in_maps = []
    b1r = np.ascontiguousarray(b1.reshape(E, F // P, P).transpose(0, 2, 1))
    b2r = np.ascontiguousarray(b2.reshape(E, H // P, P).transpose(0, 2, 1))
    for e in range(E):
        te = toks[e]
        XT = np.zeros((H, C), np.float32)
        if len(te):
            XT[:, :len(te)] = x[te].T
        in_maps.append({
            "xt": XT,
            "w1": np.ascontiguousarray(w1[e], dtype=np.float32),
            "w2": np.ascontiguousarray(w2[e], dtype=np.float32),
            "b1": b1r[e].astype(np.float32),
            "b2": b2r[e].astype(np.float32),
        })
    return in_maps


def _combine(results, toks, ps, counts):
    """out[t] = sum over the K contributions of token t."""
    contribs = []
    tok_all = []
    for e in range(E):
        cnt = int(counts[e])
        Y = results[e]["yt"][:, :cnt].T          # [cnt, H] fp32
        contribs.append(ps[e][:, None].astype(np.float32) * Y)
        tok_all.append(toks[e])
    contrib_all = np.concatenate(contribs, axis=0)
    tok_all = np.concatenate(tok_all)
    order = np.argsort(tok_all, kind="stable")
    sc = contrib_all[order]
    out = sc[0::KTOP].copy()
    for k in range(1, KTOP):
        out += sc[k::KTOP]
    return out


def kernel(hidden_states, w_router, w1, b1, w2, b2):
    x = np.ascontiguousarray(np.asarray(hidden_states, np.float32).reshape(T, H))
    w_router = np.asarray(w_router, np.float32)
    w1 = np.asarray(w1, np.float32)
    b1 = np.asarray(b1, np.float32)
    w2 = np.asarray(w2, np.float32)
    b2 = np.asarray(b2, np.float32)

    p64, p32, idx, route_probs = _route(x, w_router)
    aux = _aux_loss(p64, idx, route_probs)
    toks, ps, counts = _dispatch(x, idx, route_probs)

    C = max(P, int(-(-counts.max() // P)) * P)
    subs = _decompose(C)
    nc = _get_program(C, subs)

    in_maps = _make_in_maps(x, toks, w1, b1, w2, b2, C)
    res = _run_device(nc, in_maps)
    out = _combine(res.results, toks, ps, counts)

    return (
        out.reshape(B, S, H),
        aux,
        route_probs.reshape(B, S, KTOP),
        idx.reshape(B, S, KTOP).astype(np.int32),
    )
